# revision 1
# baseline (speedup 1.0000x reference)
"""GANLoss kernel for Trainium2: out = -sum_i prob[i, target[i]] * reward[i].

Shapes: prob (8192, 32000) f32, target (8192,) int64, reward (8192,) f32.
Sharding: rows split across 8 NeuronCores (1024 rows/core).

Strategy: the loss touches only one element per row, so instead of streaming
the full 131 MB/core shard we dma_gather the 256-float (1 KB) chunk that
contains each row's target element (4 gather calls x 256 indices per core,
~1 MB read/core), then select the element with an iota/is_equal mask fused
with the reward multiply, and reduce on the vector engine. Each core emits
a [128, 4] tile of partial sums; the host sums them and negates.
"""

import numpy as np

N, C = 8192, 32000
N_CORES = 8
ROWS_PER_CORE = N // N_CORES          # 1024
N_GATHER = 4                          # gather calls per core
ROWS_PER_CALL = 256                   # idxs per gather call
ELEM = 256                            # f32 per gathered chunk (1 KB)
CHUNKS_PER_ROW = C // ELEM            # 125; max idx 255*125+124 = 31999 < 2^15

_cached = None


def _build_bass():
    import concourse.bacc as bacc
    import concourse.mybir as mybir

    f32 = mybir.dt.float32
    i16 = mybir.dt.int16

    nc = bacc.Bacc(num_swdge_queues=4)
    prob_d = nc.declare_dram_parameter("prob", [ROWS_PER_CORE, C], f32, isOutput=False)
    gidx_d = nc.declare_dram_parameter("gidx", [128, 16 * N_GATHER], i16, isOutput=False)
    offs_d = nc.declare_dram_parameter("offs", [128, 2 * N_GATHER], f32, isOutput=False)
    rew_d = nc.declare_dram_parameter("rew", [128, 2 * N_GATHER], f32, isOutput=False)
    out_d = nc.declare_dram_parameter("out", [128, N_GATHER], f32, isOutput=True)

    with (
        nc.sbuf_tensor([128, 16 * N_GATHER], i16) as idx_sb,
        nc.sbuf_tensor([128, 2 * N_GATHER], f32) as offs_sb,
        nc.sbuf_tensor([128, 2 * N_GATHER], f32) as rew_sb,
        nc.sbuf_tensor([128, ELEM], f32) as iota_sb,
        nc.sbuf_tensor([128, N_GATHER, 2, ELEM], f32) as gath_sb,
        nc.sbuf_tensor([128, 2 * ELEM], f32) as mask_sb,
        nc.sbuf_tensor([128, 2 * ELEM], f32) as prod_sb,
        nc.sbuf_tensor([128, N_GATHER], f32) as out_sb,
        nc.semaphore("in_sem") as in_sem,
        nc.semaphore("gs0") as gs0,
        nc.semaphore("gs1") as gs1,
        nc.semaphore("gs2") as gs2,
        nc.semaphore("gs3") as gs3,
        nc.semaphore("comp_sem") as comp_sem,
        nc.semaphore("ts_sem") as ts_sem,
        nc.semaphore("iota_sem") as iota_sem,
        nc.Block() as block,
    ):
        gsems = [gs0, gs1, gs2, gs3]

        @block.gpsimd
        def _(g):
            g.iota(
                iota_sb[:],
                pattern=[[1, ELEM]],
                base=0,
                channel_multiplier=0,
                allow_small_or_imprecise_dtypes=True,
            ).then_inc(iota_sem, 1)
            g.dma_start(idx_sb[:], gidx_d[:]).then_inc(in_sem, 16)
            g.dma_start(offs_sb[:], offs_d[:]).then_inc(in_sem, 16)
            g.dma_start(rew_sb[:], rew_d[:]).then_inc(in_sem, 16)
            g.wait_ge(in_sem, 48)
            for gi in range(N_GATHER):
                src = prob_d[ROWS_PER_CALL * gi : ROWS_PER_CALL * (gi + 1), :].rearrange(
                    "r (c e) -> (r c) e", e=ELEM
                )
                g.dma_gather(
                    gath_sb[:, gi],
                    src,
                    idx_sb[:, 16 * gi : 16 * (gi + 1)],
                    num_idxs=ROWS_PER_CALL,
                    num_idxs_reg=ROWS_PER_CALL,
                    elem_size=ELEM,
                    queue_num=gi,
                ).then_inc(gsems[gi], 16)
            g.wait_ge(comp_sem, N_GATHER)
            g.dma_start(out_d[:], out_sb[:]).then_inc(in_sem, 16)
            g.wait_ge(in_sem, 64)

        @block.vector
        def _(v):
            v.wait_ge(iota_sem, 1)
            v.wait_ge(in_sem, 48)
            for gi in range(N_GATHER):
                if gi > 0:
                    v.wait_ge(ts_sem, 3 * gi)  # prior mult done: mask free
                    v.wait_ge(comp_sem, gi)  # prior reduce done: prod free
                # maskrew[p, c*ELEM + w] = (w == t%ELEM) * reward  for row 256gi+128c+p
                v.tensor_scalar(
                    mask_sb[:, 0:ELEM],
                    iota_sb[:],
                    offs_sb[:, 2 * gi : 2 * gi + 1],
                    rew_sb[:, 2 * gi : 2 * gi + 1],
                    op0=mybir.AluOpType.is_equal,
                    op1=mybir.AluOpType.mult,
                ).then_inc(ts_sem, 1)
                v.tensor_scalar(
                    mask_sb[:, ELEM : 2 * ELEM],
                    iota_sb[:],
                    offs_sb[:, 2 * gi + 1 : 2 * gi + 2],
                    rew_sb[:, 2 * gi + 1 : 2 * gi + 2],
                    op0=mybir.AluOpType.is_equal,
                    op1=mybir.AluOpType.mult,
                ).then_inc(ts_sem, 1)
                v.wait_ge(ts_sem, 3 * gi + 2)
                v.wait_ge(gsems[gi], 16)
                v.tensor_mul(
                    prod_sb[:],
                    gath_sb[:, gi].rearrange("p a b -> p (a b)"),
                    mask_sb[:],
                ).then_inc(ts_sem, 1)
                v.wait_ge(ts_sem, 3 * gi + 3)
                v.tensor_reduce(
                    out_sb[:, gi : gi + 1],
                    prod_sb[:],
                    axis=mybir.AxisListType.X,
                    op=mybir.AluOpType.add,
                ).then_inc(comp_sem, 1)

    nc.compile()
    return nc


def _shard_host_inputs(prob, target, reward):
    """Per-core in_maps: prob shard + precomputed gather indices/offsets."""
    t_all = np.asarray(target).astype(np.int64)
    r_all = np.asarray(reward).astype(np.float32)
    prob = np.ascontiguousarray(np.asarray(prob, dtype=np.float32))
    in_maps = []
    loc = np.arange(ROWS_PER_CALL)
    for core in range(N_CORES):
        base = core * ROWS_PER_CORE
        t = t_all[base : base + ROWS_PER_CORE]
        r = r_all[base : base + ROWS_PER_CORE]
        chunk = (t // ELEM).astype(np.int64)
        off = (t % ELEM).astype(np.float32)
        gidx16 = np.zeros((16, 16 * N_GATHER), np.int16)
        offs = np.zeros((128, 2 * N_GATHER), np.float32)
        rew = np.zeros((128, 2 * N_GATHER), np.float32)
        for g in range(N_GATHER):
            rb = ROWS_PER_CALL * g
            idxv = loc * CHUNKS_PER_ROW + chunk[rb + loc]
            gidx16[loc % 16, 16 * g + loc // 16] = idxv.astype(np.int16)
            for ci in range(2):
                offs[:, 2 * g + ci] = off[rb + 128 * ci : rb + 128 * ci + 128]
                rew[:, 2 * g + ci] = r[rb + 128 * ci : rb + 128 * ci + 128]
        # the 8 GPSIMD cores each read their own 16-partition copy
        gidx = np.tile(gidx16, (8, 1))
        in_maps.append(
            {
                "prob": prob[base : base + ROWS_PER_CORE],
                "gidx": gidx,
                "offs": offs,
                "rew": rew,
            }
        )
    return in_maps


def kernel(prob, target, reward):
    global _cached
    from concourse.bass_utils import run_bass_kernel_spmd

    if _cached is None:
        _cached = _build_bass()
    nc = _cached
    in_maps = _shard_host_inputs(prob, target, reward)
    res = run_bass_kernel_spmd(nc, in_maps, list(range(N_CORES)))
    total = np.float64(0.0)
    for core_out in res.results:
        total += np.asarray(core_out["out"], dtype=np.float64).sum()
    return np.float32(-total)



# revision 4
# speedup vs baseline: 2.3352x; 2.3352x over previous
"""GANLoss kernel for Trainium2: out = -sum_i prob[i, target[i]] * reward[i].

Shapes: prob (8192, 32000) f32, target (8192,) int64, reward (8192,) f32.
Sharding: rows split across 8 NeuronCores (1024 rows/core).

Strategy: the loss touches one element per row, so each core runs a single
gpsimd indirect DMA (qPoolDynamic) that gathers the 1024 target elements
straight out of the DRAM prob shard using host-precomputed flat int32
offsets that the descriptor generator reads directly from DRAM (no staging
DMA for the indices). In parallel the sync engine DMAs the reward tile into
SBUF. One fused DVE tensor_tensor_reduce computes
acc[p] = -sum_j gathered[p,j]*reward[p,j], and the sync engine DMAs the
[128,1] partials out. The host sums the 8 cores' partials (already negated).
"""

import numpy as np

N, C = 8192, 32000
N_CORES = 8
ROWS_PER_CORE = N // N_CORES          # 1024
P = 128                               # SBUF partitions
J = ROWS_PER_CORE // P                # 8 gathered elements per partition

_cached = None


def _build_bass():
    import concourse.bacc as bacc
    import concourse.bass as bass
    import concourse.mybir as mybir

    f32 = mybir.dt.float32
    i32 = mybir.dt.int32

    nc = bacc.Bacc()
    prob_d = nc.declare_dram_parameter("prob", [ROWS_PER_CORE, C], f32, isOutput=False)
    idx_d = nc.declare_dram_parameter("gidx", [P, J], i32, isOutput=False)
    rew_d = nc.declare_dram_parameter("rew", [P, J], f32, isOutput=False)
    out_d = nc.declare_dram_parameter("out", [P, 1], f32, isOutput=True)

    with (
        nc.sbuf_tensor([P, J], f32) as gath_sb,
        nc.sbuf_tensor([P, J], f32) as rew_sb,
        nc.sbuf_tensor([P, J], f32) as prod_sb,
        nc.sbuf_tensor([P, 1], f32) as acc_sb,
        nc.semaphore("rsem") as rsem,
        nc.semaphore("gsem") as gsem,
        nc.semaphore("vsem") as vsem,
        nc.semaphore("osem") as osem,
        nc.Block() as block,
    ):

        @block.sync
        def _(s):
            s.dma_start(rew_sb[:], rew_d[:]).then_inc(rsem, 16)
            s.wait_ge(vsem, 1)
            s.dma_start(out_d[:], acc_sb[:]).then_inc(osem, 16)

        @block.gpsimd
        def _(g):
            g.indirect_dma_start(
                out=gath_sb[:],
                out_offset=None,
                in_=prob_d[:],
                in_offset=bass.IndirectOffsetOnAxis(ap=idx_d[:], axis=1),
            ).then_inc(gsem, 16)

        @block.vector
        def _(v):
            v.wait_ge(gsem, 16)
            v.wait_ge(rsem, 16)
            v.tensor_tensor_reduce(
                out=prod_sb[:],
                in0=gath_sb[:],
                in1=rew_sb[:],
                scale=-1.0,
                scalar=0.0,
                op0=mybir.AluOpType.mult,
                op1=mybir.AluOpType.add,
                accum_out=acc_sb[:],
            ).then_inc(vsem, 1)

    nc.compile()
    return nc


def _shard_host_inputs(prob, target, reward):
    """Per-core in_maps: prob shard + flat int32 gather offsets + reward tile."""
    t_all = np.asarray(target).astype(np.int64)
    r_all = np.asarray(reward).astype(np.float32)
    prob = np.ascontiguousarray(np.asarray(prob, dtype=np.float32))
    row_base = np.arange(ROWS_PER_CORE, dtype=np.int64) * C
    in_maps = []
    for core in range(N_CORES):
        base = core * ROWS_PER_CORE
        flat = row_base + t_all[base : base + ROWS_PER_CORE]
        in_maps.append(
            {
                "prob": prob[base : base + ROWS_PER_CORE],
                "gidx": flat.astype(np.int32).reshape(P, J),
                "rew": r_all[base : base + ROWS_PER_CORE].reshape(P, J),
            }
        )
    return in_maps


def kernel(prob, target, reward):
    global _cached
    from concourse.bass_utils import run_bass_kernel_spmd

    if _cached is None:
        _cached = _build_bass()
    nc = _cached
    in_maps = _shard_host_inputs(prob, target, reward)
    res = run_bass_kernel_spmd(nc, in_maps, list(range(N_CORES)))
    total = np.float64(0.0)
    for core_out in res.results:
        total += np.asarray(core_out["out"], dtype=np.float64).sum()
    return np.float32(total)


# revision 10
# speedup vs baseline: 3.9165x; 1.6772x over previous
"""GANLoss kernel for Trainium2: out = -sum_i prob[i, target[i]] * reward[i].

Shapes: prob (8192, 32000) f32, target (8192,) int64, reward (8192,) f32.
Sharding: rows split across 8 NeuronCores (1024 rows/core).

Strategy (per core, all on the gpsimd/Pool engine):
 1. One DMA stages a [128, 20] int32 metadata tile into SBUF: 16 flat
    gather offsets per partition plus the scatter's identity int16 indices
    (wrapped in 16 partitions, replicated for the 8 Q7 cores).
 2. The host appends reward as a 32001st column of the prob shard, so ONE
    qPoolDynamic indirect DMA gathers the 8 target elements and 8 rewards
    per partition (2048 x 4B total) from the DRAM shard into [128, 16]
    SBUF (cols 0:8 picks, cols 8:16 matching rewards).
 3. A gpsimd tensor_tensor multiplies picks by rewards -> prod [128, 8].
 4. A dma_scatter_add (128 identity indices, elem 8, row stride 64) lands
    each partition's products in its own zero-initialized output row.
The host sums the 8 cores' [128, 64] partials (only cols 0:8 nonzero)
and negates.
"""

import numpy as np

N, C = 8192, 32000
CX = C + 1                            # prob row + appended reward column
N_CORES = 8
ROWS_PER_CORE = N // N_CORES          # 1024
P = 128                               # partitions
J = ROWS_PER_CORE // P                # 8 rows per partition
MW = 2 * J + J // 2                   # meta width: 16 i32 offsets + 8 i16 idx
OSTRIDE = 64                          # scatter elem_step (256B row stride)

_cached = None


def _build_bass():
    import concourse.bacc as bacc
    import concourse.bass as bass
    import concourse.mybir as mybir
    from concourse import library_config

    f32 = mybir.dt.float32
    i32 = mybir.dt.int32
    i16 = mybir.dt.int16

    nc = bacc.Bacc()
    prob_d = nc.declare_dram_parameter("prob", [ROWS_PER_CORE, CX], f32, isOutput=False)
    meta_d = nc.declare_dram_parameter("meta", [P, MW], i32, isOutput=False)
    out_d = nc.declare_dram_parameter("out", [P, OSTRIDE], f32, isOutput=True)

    with (
        nc.sbuf_tensor([P, MW], i32) as meta_sb,
        nc.sbuf_tensor([P, 2 * J], f32) as gath_sb,
        nc.sbuf_tensor([P, 1, J], f32) as prod_sb,
        nc.semaphore("lsem") as lsem,
        nc.semaphore("gsem") as gsem,
        nc.semaphore("vsem") as vsem,
        nc.semaphore("osem") as osem,
    ):
        g = nc.gpsimd
        g.dma_start(meta_sb[:], meta_d[:]).then_inc(lsem, 16)
        g.wait_ge(lsem, 16)
        g.indirect_dma_start(
            out=gath_sb[:],
            out_offset=None,
            in_=prob_d[:],
            in_offset=bass.IndirectOffsetOnAxis(ap=meta_sb[:, 0 : 2 * J], axis=1),
        ).then_inc(gsem, 16)
        g.wait_ge(gsem, 16)
        g.tensor_tensor(
            out=prod_sb[:, 0, :],
            in0=gath_sb[:, 0:J],
            in1=gath_sb[:, J : 2 * J],
            op=mybir.AluOpType.mult,
        ).then_inc(vsem, 1)
        g.load_library(library_config.mlp)
        g.wait_ge(vsem, 1)
        g.dma_scatter_add(
            out_ap=out_d[:, 0:J],
            in_ap=prod_sb[:],
            idxs_ap=meta_sb[:, 2 * J : MW].bitcast(i16),
            num_idxs=P,
            num_idxs_reg=P,
            elem_size=J,
            elem_step=OSTRIDE,
        ).then_inc(osem, 16)
        g.wait_ge(osem, 16)

    nc.compile()
    return nc


def _shard_host_inputs(prob, target, reward):
    """Per-core in_maps: prob shard with reward column + packed metadata."""
    t_all = np.asarray(target).astype(np.int64)
    r_all = np.asarray(reward).astype(np.float32)
    prob = np.asarray(prob, dtype=np.float32)
    rows = np.arange(ROWS_PER_CORE, dtype=np.int64)

    # Scatter identity indices, wrapped: value(ch, s) = s*16 + ch so the
    # unwrap (s p) ordering yields 0..127; tiled for the 8 Q7 core replicas.
    ch, s = np.meshgrid(np.arange(16), np.arange(J), indexing="ij")
    sidx16 = (s * 16 + ch).astype(np.int16)                     # (16, 8)
    sidx = np.tile(sidx16, (8, 1)).view(np.int32)               # (128, 4) as i32

    in_maps = []
    for core in range(N_CORES):
        base = core * ROWS_PER_CORE
        probx = np.concatenate(
            [prob[base : base + ROWS_PER_CORE],
             r_all[base : base + ROWS_PER_CORE, None]],
            axis=1,
        )
        pick = (rows * CX + t_all[base : base + ROWS_PER_CORE]).astype(np.int32)
        rcol = (rows * CX + C).astype(np.int32)
        meta = np.concatenate(
            [pick.reshape(P, J), rcol.reshape(P, J), sidx], axis=1
        )                                                       # (128, 20) i32
        # "out" seeds the zero-initialized output buffer for simulators that
        # mark unwritten memory (the PJRT/NRT runtimes donate zeroed buffers);
        # runners that only bind declared ExternalInputs ignore this key.
        in_maps.append(
            {
                "prob": probx,
                "meta": meta,
                "out": np.zeros((P, OSTRIDE), np.float32),
            }
        )
    return in_maps


def kernel(prob, target, reward):
    global _cached
    from concourse.bass_utils import run_bass_kernel_spmd

    if _cached is None:
        _cached = _build_bass()
    nc = _cached
    in_maps = _shard_host_inputs(prob, target, reward)
    res = run_bass_kernel_spmd(nc, in_maps, list(range(N_CORES)))
    total = np.float64(0.0)
    for core_out in res.results:
        total += np.asarray(core_out["out"], dtype=np.float64).sum()
    return np.float32(-total)


# revision 12
# speedup vs baseline: 4.4541x; 1.1372x over previous
"""GANLoss kernel for Trainium2: out = -sum_i prob[i, target[i]] * reward[i].

Shapes: prob (8192, 32000) f32, target (8192,) int64, reward (8192,) f32.
Sharding: rows split across 8 NeuronCores (1024 rows/core).

Per-core pipeline (all on the gpsimd/Pool engine):
 1. One DMA stages a [128, 336] uint16 metadata tile into SBUF: per-call
    int16 gather indices, per-group uint16 select positions, a reward-valued
    one-hot mask (f32), and identity int16 scatter indices.
 2. 8 dma_gather calls fetch, for each of the 1024 rows, the 512B chunk of
    its prob row containing the target element (row 128g+p -> partition p,
    slot g; chunk index = p*250 + target//128, int16-safe).
 3. indirect_copy selects 128 candidate elements per partition (each
    16-partition group shares its union index list; a partition's own picks
    sit at columns i with i%16 == p%16).
 4. tensor_tensor multiplies by the mask, which holds reward at own-pick
    cells and 0 elsewhere - fusing the reward multiply with junk removal.
 5. dma_scatter_add (identity indices, elem 128) lands each partition's
    128-wide masked products in its own row of the zeroed output.
The host sums the 8 cores' [128, 128] partials (junk cells are exact
zeros) and negates.
"""

import numpy as np

N, C = 8192, 32000
N_CORES = 8
ROWS_PER_CORE = N // N_CORES          # 1024
P = 128                               # partitions
S = ROWS_PER_CORE // P                # 8 row-slots per partition / gather calls
ELEM = 128                            # chunk width (512B)
CPR = C // ELEM                       # 250 chunks per row
MW = 336                              # meta width in uint16

# meta layout (uint16 columns)
GIDX0, GIDX1 = 0, 8 * S               # 8 calls x [128, 8] int16
CIDX0, CIDX1 = 64, 72                 # icopy positions, uint16
MASK0, MASK1 = 72, 328                # [128, 128] f32 reward mask
SIDX0, SIDX1 = 328, 336               # scatter identity, int16

_cached = None


def _build_bass():
    import concourse.bacc as bacc
    import concourse.mybir as mybir
    from concourse import library_config

    f32 = mybir.dt.float32
    i16 = mybir.dt.int16
    u16 = mybir.dt.uint16

    nc = bacc.Bacc()
    prob_d = nc.declare_dram_parameter("prob", [ROWS_PER_CORE, C], f32, isOutput=False)
    meta_d = nc.declare_dram_parameter("meta", [P, MW], u16, isOutput=False)
    out_d = nc.declare_dram_parameter("out", [P, ELEM], f32, isOutput=True)

    with (
        nc.sbuf_tensor([P, MW], u16) as meta_sb,
        nc.sbuf_tensor([P, S, 1, ELEM], f32) as gath_sb,
        nc.sbuf_tensor([P, ELEM], f32) as sel_sb,
        nc.sbuf_tensor([P, 1, ELEM], f32) as prod_sb,
        nc.semaphore("lsem") as lsem,
        nc.semaphore("gsem") as gsem,
        nc.semaphore("csem") as csem,
        nc.semaphore("vsem") as vsem,
        nc.semaphore("osem") as osem,
    ):
        g = nc.gpsimd
        g.dma_start(meta_sb[:], meta_d[:]).then_inc(lsem, 16)
        g.wait_ge(lsem, 16)
        g.load_library(library_config.mlp)
        for call in range(S):
            src = prob_d[P * call : P * (call + 1), :].rearrange(
                "r (c e) -> (r c) e", e=ELEM
            )
            g.dma_gather(
                gath_sb[:, call],
                src,
                meta_sb[:, 8 * call : 8 * (call + 1)].bitcast(i16),
                num_idxs=P,
                num_idxs_reg=P,
                elem_size=ELEM,
            ).then_inc(gsem, 16)
        g.wait_ge(gsem, 16 * S)
        g.indirect_copy(
            sel_sb[:],
            gath_sb[:].rearrange("p a b c -> p (a b c)"),
            meta_sb[:, CIDX0:CIDX1],
            i_know_ap_gather_is_preferred=True,
        ).then_inc(csem, 1)
        g.load_library(library_config.standard)
        g.wait_ge(csem, 1)
        g.tensor_tensor(
            out=prod_sb[:, 0, :],
            in0=sel_sb[:],
            in1=meta_sb[:, MASK0:MASK1].bitcast(f32),
            op=mybir.AluOpType.mult,
        ).then_inc(vsem, 1)
        g.load_library(library_config.mlp)
        g.wait_ge(vsem, 1)
        g.dma_scatter_add(
            out_ap=out_d[:],
            in_ap=prod_sb[:],
            idxs_ap=meta_sb[:, SIDX0:SIDX1].bitcast(i16),
            num_idxs=P,
            num_idxs_reg=P,
            elem_size=ELEM,
        ).then_inc(osem, 16)
        g.wait_ge(osem, 16)

    nc.compile()
    return nc


def _shard_host_inputs(prob, target, reward):
    """Per-core in_maps: prob shard + packed uint16 metadata tile."""
    t_all = np.asarray(target).astype(np.int64)
    r_all = np.asarray(reward).astype(np.float32)
    prob = np.asarray(prob, dtype=np.float32)

    # identity scatter indices, wrapped (value(ch, s) = s*16 + ch), tiled x8
    ch, s = np.meshgrid(np.arange(16), np.arange(S), indexing="ij")
    ident16 = (s * 16 + ch).astype(np.int16)                    # (16, 8)
    sidx_u16 = np.tile(ident16, (8, 1)).view(np.uint16)         # (128, 8)

    in_maps = []
    for core in range(N_CORES):
        base = core * ROWS_PER_CORE
        t = t_all[base : base + ROWS_PER_CORE]                  # (1024,)
        r = r_all[base : base + ROWS_PER_CORE]

        meta = np.zeros((P, MW), np.uint16)

        # gather indices: call g, idx# k -> partition k holds row 128g+k's
        # chunk; wrapped [16, 8] with value(ch, s2) = idx#(s2*16+ch), tiled x8
        for call in range(S):
            rows = t[P * call : P * (call + 1)]                 # targets of rows 128g+k
            idxv = (np.arange(P) * CPR + rows // ELEM).astype(np.int16)  # (128,)
            wrapped = idxv.reshape(S, 16).T                     # [ch, s2] = idx#(s2*16+ch)
            meta[:, 8 * call : 8 * (call + 1)] = np.tile(
                wrapped.view(np.uint16), (8, 1)
            )

        # icopy positions: group q's list item i = s*128 + t(128s+16q+i%16)%128
        # (s = i//16); stored wrapped: value(ch2, s2) at row 16q+ch2, col s2
        cidx = np.zeros((P, S), np.uint16)
        for q in range(8):
            for ch2 in range(16):
                for s2 in range(S):
                    row = 128 * s2 + 16 * q + ch2
                    cidx[16 * q + ch2, s2] = s2 * ELEM + (t[row] % ELEM)
        meta[:, CIDX0:CIDX1] = cidx

        # reward-valued one-hot mask [128, 128] f32
        mask = np.zeros((P, ELEM), np.float32)
        pp, ii = np.meshgrid(np.arange(P), np.arange(ELEM), indexing="ij")
        own = (ii % 16) == (pp % 16)
        rowsel = 128 * (ii // 16) + pp
        mask[own] = r[rowsel[own]]
        meta[:, MASK0:MASK1] = mask.view(np.uint16)

        meta[:, SIDX0:SIDX1] = sidx_u16

        # "out" seeds the zero-initialized output buffer for simulators that
        # mark unwritten memory (the PJRT/NRT runtimes donate zeroed buffers);
        # runners that only bind declared ExternalInputs ignore this key.
        in_maps.append(
            {
                "prob": prob[base : base + ROWS_PER_CORE],
                "meta": meta,
                "out": np.zeros((P, ELEM), np.float32),
            }
        )
    return in_maps


def kernel(prob, target, reward):
    global _cached
    from concourse.bass_utils import run_bass_kernel_spmd

    if _cached is None:
        _cached = _build_bass()
    nc = _cached
    in_maps = _shard_host_inputs(prob, target, reward)
    res = run_bass_kernel_spmd(nc, in_maps, list(range(N_CORES)))
    total = np.float64(0.0)
    for core_out in res.results:
        total += np.asarray(core_out["out"], dtype=np.float64).sum()
    return np.float32(-total)
